# revision 3
# baseline (speedup 1.0000x reference)
"""Trainium2 Bass kernel for nn_GatedAttentionUnit (Swin windowed gated attention).

Self-contained: takes FULL inputs, shards across 8 NeuronCores, returns FULL output.

Strategy
--------
The reference computes, per batch: LN -> gate/Q and K/V projections (SiLU),
Swin shifted-window attention over 16 windows (2304 tokens each) with the
standard shift mask, merge+unroll, multiplicative gate, 2-layer output MLP,
residual.

Key structural facts exploited here:
1. roll + window-split + region-sort is a pure token permutation, and every op
   outside the attention matmuls is per-token => the permutation is applied on
   the HOST to the raw inputs (a gather), and its inverse to the output.
2. The Swin shift mask makes each window's attention exactly block-diagonal
   after sorting tokens by mask region:
       win(0,0): 1x2304    win(0,1): 2x1152   win(1,0): 2x1152   win(1,1): 4x576
   so the device kernel does dense *unmasked* attention on blocks only
   (2.25/4 of the naive work, no mask tensor at all).
3. Splitting win(0,0) by query halves gives a perfectly uniform per-core shape:
   every core runs queries [1152x2304, 1152^2, 1152^2, 576^2, 576^2] = 5.97M
   score elements; 8 cores cover batch(4) x all windows exactly.
4. LayerNorm's affine (g,b) is folded into the projection weights on the host.
5. Scores are tiny (|S| ~ 1e-4 after the 1/(c*seq) scaling) so softmax needs
   no max-subtraction; the scale is folded into the Exp activation.

Device layout: channel-major ("T") tensors [128=C partitions, tokens free] for
everything except V (token-major, as the PV-matmul stationary operand).
Matmuls run as float32r (full PE rate at N>=256). Softmax row-sums accumulate
on DVE across key-tiles and reduce across partitions with a ones-matmul.
"""

import numpy as np

# ---------------------------------------------------------------- constants
B, H, W, C, NS = 4, 96, 96, 128, 2
WH, WW = H // NS, W // NS      # 48
SH, SW = WH // 2, WW // 2      # 24
SEQ = H * W                    # 9216
NQ, NKV = 4608, 5888           # per-core query / kv tokens (kv incl. 2x64 pad for 128-alignment)
SCALE = 1.0 / float(C * SEQ)   # attention score scale
EPS = 1e-5

# (q0, k0, qn, kn) — identical block list on every core
BLOCKS = [
    (0, 0, 1152, 2304),
    (1152, 2304, 1152, 1152),
    (2304, 3456, 1152, 1152),
    (3456, 4608, 576, 576),
    (4032, 5248, 576, 576),
]


def _qchunks(qn):
    if qn == 1152:
        return [384, 384, 384]
    assert qn == 576
    return [320, 256]


def _jtiles(k0, kn):
    """(joff, jlen) tiles of <=128 keys that never cross a 128-token V-tile boundary."""
    out = []
    j = k0
    end = k0 + kn
    while j < end:
        step = min(128 - (j % 128), end - j)
        out.append((j, step))
        j += step
    return out


def _win_tokens(wy, wx):
    r = np.arange(WH)[:, None]
    c = np.arange(WW)[None, :]
    oy = (WH * wy + r + SH) % H
    ox = (WW * wx + c + SW) % W
    return oy * W + ox


def _core_index_lists():
    t00, t01, t10, t11 = (_win_tokens(0, 0), _win_tokens(0, 1),
                          _win_tokens(1, 0), _win_tokens(1, 1))
    win0_h0 = t00[:SH, :].ravel()
    win0_h1 = t00[SH:, :].ravel()
    w1a, w1b = t01[:, :SW].ravel(), t01[:, SW:].ravel()
    w2a, w2b = t10[:SH, :].ravel(), t10[SH:, :].ravel()
    w3 = [t11[:SH, :SW].ravel(), t11[:SH, SW:].ravel(),
          t11[SH:, :SW].ravel(), t11[SH:, SW:].ravel()]
    q_idx = np.zeros((8, NQ), dtype=np.int64)
    kv_idx = np.zeros((8, NKV), dtype=np.int64)
    for core in range(8):
        half = core % 2
        mine, other = (win0_h0, win0_h1) if half == 0 else (win0_h1, win0_h0)
        if half == 0:
            b1, b2, b3, b4 = w1a, w1b, w3[0], w3[1]
        else:
            b1, b2, b3, b4 = w2a, w2b, w3[2], w3[3]
        pad = np.zeros(64, dtype=b3.dtype)
        kv_idx[core] = np.concatenate([mine, other, b1, b2, b3, pad, b4, pad])
        q_idx[core] = np.concatenate([mine, b1, b2, b3, b4])
    return q_idx, kv_idx


_Q_IDX, _KV_IDX = _core_index_lists()

# ---------------------------------------------------------------- device program

_PROGRAM = None  # cached (nc,) — compile once per process


def _build_program():
    import concourse.bass as bass
    import concourse.tile as tile
    from concourse import bacc, mybir

    f32 = mybir.dt.float32
    f32r = mybir.dt.float32r
    AF = mybir.ActivationFunctionType
    ts, ds = bass.ts, bass.ds

    nc = bacc.Bacc()

    # ---- DRAM parameters
    xq_d = nc.declare_dram_parameter("xq", [NQ, C], f32, isOutput=False)
    xkv_d = nc.declare_dram_parameter("xkv", [NKV, C], f32, isOutput=False)
    wgq_d = nc.declare_dram_parameter("wgq", [C, 2 * C], f32, isOutput=False)
    wkv_d = nc.declare_dram_parameter("wkv", [C, 2 * C], f32, isOutput=False)
    wo1_d = nc.declare_dram_parameter("wo1", [C, C], f32, isOutput=False)
    wo2_d = nc.declare_dram_parameter("wo2", [C, C], f32, isOutput=False)
    bgq_d = nc.declare_dram_parameter("bgq", [2 * C, 1], f32, isOutput=False)
    bkv_d = nc.declare_dram_parameter("bkv", [2 * C, 1], f32, isOutput=False)
    bo1_d = nc.declare_dram_parameter("bo1", [C, 1], f32, isOutput=False)
    bvb_d = nc.declare_dram_parameter("bvb", [C, C], f32, isOutput=False)   # V-bias row broadcast
    ident_d = nc.declare_dram_parameter("ident", [128, 128], f32, isOutput=False)
    ones_d = nc.declare_dram_parameter("onescol", [128, 1], f32, isOutput=False)
    eps_d = nc.declare_dram_parameter("epsc", [128, 1], f32, isOutput=False)
    onerow_d = nc.declare_dram_parameter("onerow", [1, 128], f32, isOutput=False)
    y_d = nc.declare_dram_parameter("y", [NQ, C], f32, isOutput=True)

    with tile.TileContext(nc) as tc:
        with (
            tc.tile_pool(name="consts", bufs=1) as cpool,
            tc.tile_pool(name="big", bufs=1) as bigpool,
            tc.tile_pool(name="xin", bufs=4) as xpool,
            tc.tile_pool(name="xnorm", bufs=4) as xnpool,
            tc.tile_pool(name="stats", bufs=6) as spool,
            tc.tile_pool(name="esb", bufs=4) as epool,
            tc.tile_pool(name="racc", bufs=2) as rpool,
            tc.tile_pool(name="small1", bufs=2) as onepool,
            tc.tile_pool(name="t1", bufs=4) as tpool,
            tc.tile_pool(name="yout", bufs=4) as ypool,
            tc.tile_pool(name="ps", bufs=3, space="PSUM") as pspool,
            tc.tile_pool(name="psO", bufs=2, space="PSUM") as opool,
            tc.tile_pool(name="psS", bufs=2, space="PSUM") as rspool,
        ):
            # ---- constants into SBUF
            def cdma(shape, src, tag, dt=f32):
                t = cpool.tile(shape, dt, tag=tag)
                nc.sync.dma_start(t[:], src.bitcast(dt) if dt is not f32 else src)
                return t

            wgq = cdma([C, 2 * C], wgq_d[:], "wgq", f32r)
            wkv = cdma([C, 2 * C], wkv_d[:], "wkv", f32r)
            wo1 = cdma([C, C], wo1_d[:], "wo1", f32r)
            wo2 = cdma([C, C], wo2_d[:], "wo2", f32r)
            bg = cdma([C, 1], bgq_d[0:C, :], "bg")
            bq = cdma([C, 1], bgq_d[C:2 * C, :], "bq")
            bk = cdma([C, 1], bkv_d[0:C, :], "bk")
            bo1 = cdma([C, 1], bo1_d[:], "bo1")
            bvb = cdma([C, C], bvb_d[:], "bvb")
            ident = cdma([128, 128], ident_d[:], "ident")
            onescol = cdma([128, 1], ones_d[:], "onescol")
            epsc = cdma([128, 1], eps_d[:], "epsc")
            onerow = cdma([1, 128], onerow_d[:], "onerow")

            # ---- big persistent SBUF tensors (tags shared across phases to save SBUF)
            XqT = bigpool.tile([C, NQ], f32r, tag="bigA")     # later reused as OgT
            XkvT = bigpool.tile([C, NKV], f32r, tag="bigB")   # later reused as HT
            QT = bigpool.tile([C, NQ], f32r, tag="bigC")      # later reused as Y2T
            KT = bigpool.tile([C, NKV], f32r, tag="KT")
            GT = bigpool.tile([C, NQ], f32, tag="GT")
            V = bigpool.tile([128, NKV], f32r, tag="V")       # token-major, 45 tiles of [128,128]

            # ---- phase 1: load + layernorm (sans affine) + transpose
            def ln_transpose(x_dram, n_tok, XT):
                for t in range(n_tok // 128):
                    x = xpool.tile([128, C], f32, tag="x")
                    nc.sync.dma_start(x[:], x_dram[ts(t, 128), :])
                    # stats on ACT: sum(x) and sum(x^2) via accum_out
                    dump = xnpool.tile([128, C], f32, tag="dump")
                    msum = spool.tile([128, 1], f32, tag="msum")
                    s2 = spool.tile([128, 1], f32, tag="s2")
                    nc.scalar.activation(dump[:], x[:], AF.Copy, accum_out=msum[:])
                    nc.scalar.activation(dump[:], x[:], AF.Square, accum_out=s2[:])
                    m = spool.tile([128, 1], f32, tag="m")
                    nc.scalar.mul(m[:], msum[:], 1.0 / C)
                    m2 = spool.tile([128, 1], f32, tag="m2")
                    nc.vector.tensor_mul(m2[:], m[:], m[:])
                    var = spool.tile([128, 1], f32, tag="var")
                    nc.vector.tensor_scalar(var[:], s2[:], 1.0 / C, m2[:],
                                            mybir.AluOpType.mult,
                                            mybir.AluOpType.subtract)
                    std = spool.tile([128, 1], f32, tag="std")
                    nc.scalar.activation(std[:], var[:], AF.Sqrt, bias=epsc[:])
                    rstd = spool.tile([128, 1], f32, tag="rstd")
                    nc.vector.reciprocal(rstd[:], std[:])
                    xn = xnpool.tile([128, C], f32, tag="xn")
                    nc.vector.tensor_scalar(xn[:], x[:], m[:], rstd[:],
                                            mybir.AluOpType.subtract,
                                            mybir.AluOpType.mult)
                    tr = pspool.tile([128, 128], f32, tag="ps")
                    nc.tensor.transpose(tr[:], xn[:], ident[:])
                    nc.scalar.copy(XT[:, ts(t, 128)], tr[:])

            ln_transpose(xq_d, NQ, XqT)
            ln_transpose(xkv_d, NKV, XkvT)

            # ---- phase 2: projections
            def proj(wT, XT, n_tok, bias, outT, act=AF.Silu):
                off = 0
                while off < n_tok:
                    n = min(512, n_tok - off)
                    ps = pspool.tile([128, 512], f32, tag="ps")
                    nc.tensor.matmul(ps[:, 0:n], wT,
                                     XT[:, ds(off, n)],
                                     start=True, stop=True)
                    nc.scalar.activation(outT[:, ds(off, n)], ps[:, 0:n], act, bias=bias[:])
                    off += n

            proj(wgq[:, 0:C], XqT, NQ, bg, GT)          # gate (channel-major)
            proj(wgq[:, C:2 * C], XqT, NQ, bq, QT)      # Q
            proj(wkv[:, 0:C], XkvT, NKV, bk, KT)        # K
            # V token-major via per-tile Form A: lhsT = XkvT tile, rhs = wkv (both halves,
            # keep only the V half); bias is along the free axis -> DVE add then SiLU.
            for t in range(NKV // 128):
                ps = pspool.tile([128, 2 * C], f32, tag="ps")
                nc.tensor.matmul(ps[:], XkvT[:, ts(t, 128)],
                                 wkv, start=True, stop=True)
                vt = tpool.tile([128, C], f32, tag="vtmp")
                nc.vector.tensor_add(vt[:], ps[:, C:2 * C], bvb[:])
                nc.scalar.activation(V[:, ts(t, 128)], vt[:], AF.Silu)

            # ---- phase 3: blockwise attention -> OgT = (softmax(S) @ V)^T * rinv * gate
            OgT = bigpool.tile([C, NQ], f32r, tag="bigA")  # reuses XqT slot
            for (q0, k0, qn, kn) in BLOCKS:
                jt = _jtiles(k0, kn)
                qc_off = 0
                for qcn in _qchunks(qn):
                    qs = q0 + qc_off
                    o_ps = opool.tile([128, 384], f32, tag="O")
                    racc = rpool.tile([128, 384], f32, tag="racc")
                    nc.gpsimd.memset(racc[:, 0:qcn], 0.0)
                    for ji, (joff, jlen) in enumerate(jt):
                        vt_i, p0 = joff // 128, joff % 128
                        s_ps = pspool.tile([128, 384], f32, tag="ps")
                        nc.tensor.matmul(s_ps[p0:p0 + jlen, 0:qcn],
                                         KT[:, ds(joff, jlen)],
                                         QT[:, ds(qs, qcn)],
                                         start=True, stop=True)
                        e = epool.tile([128, 384], f32r, tag="e")
                        nc.scalar.activation(e[p0:p0 + jlen, 0:qcn],
                                             s_ps[p0:p0 + jlen, 0:qcn],
                                             AF.Exp, scale=SCALE)
                        nc.vector.tensor_add(racc[p0:p0 + jlen, 0:qcn],
                                             racc[p0:p0 + jlen, 0:qcn],
                                             e[p0:p0 + jlen, 0:qcn])
                        nc.tensor.matmul(o_ps[:, 0:qcn],
                                         V[p0:p0 + jlen, ts(vt_i, 128)],
                                         e[p0:p0 + jlen, 0:qcn],
                                         start=(ji == 0), stop=(ji == len(jt) - 1))
                    # rowsum across partitions via ones-matmul, reciprocal, broadcast
                    rs_ps = rspool.tile([1, 384], f32, tag="rs")
                    nc.tensor.matmul(rs_ps[:, 0:qcn], onescol[:],
                                     racc[:, 0:qcn], start=True, stop=True)
                    rinv = onepool.tile([1, 384], f32, tag="rinv")
                    nc.vector.reciprocal(rinv[:, 0:qcn], rs_ps[:, 0:qcn])
                    rb_ps = pspool.tile([128, 384], f32, tag="ps")
                    nc.tensor.matmul(rb_ps[:, 0:qcn], onerow[:],
                                     rinv[:, 0:qcn], start=True, stop=True)
                    t1 = tpool.tile([128, 384], f32, tag="t1")
                    nc.vector.tensor_mul(t1[:, 0:qcn], o_ps[:, 0:qcn], GT[:, ds(qs, qcn)])
                    nc.vector.tensor_mul(OgT[:, ds(qs, qcn)], t1[:, 0:qcn], rb_ps[:, 0:qcn])
                    qc_off += qcn

            # ---- phase 4: output MLP (channel-major) + transpose + residual + store
            HT = bigpool.tile([C, NQ], f32r, tag="bigB")
            proj(wo1, OgT, NQ, bo1, HT)
            Y2T = bigpool.tile([C, NQ], f32, tag="bigC")
            off = 0
            while off < NQ:
                n = min(512, NQ - off)
                ps = pspool.tile([128, 512], f32, tag="ps")
                nc.tensor.matmul(ps[:, 0:n], wo2,
                                 HT[:, ds(off, n)],
                                 start=True, stop=True)
                nc.scalar.copy(Y2T[:, ds(off, n)], ps[:, 0:n])
                off += n
            for t in range(NQ // 128):
                xr = xpool.tile([128, C], f32, tag="x")
                nc.sync.dma_start(xr[:], xq_d[ts(t, 128), :])
                tr = pspool.tile([128, 128], f32, tag="ps")
                nc.tensor.transpose(tr[:], Y2T[:, ts(t, 128)], ident[:])
                yt = ypool.tile([128, C], f32, tag="yt")
                nc.vector.tensor_add(yt[:], tr[:], xr[:])
                nc.sync.dma_start(y_d[ts(t, 128), :], yt[:])

    nc.compile()
    return nc


def _get_program():
    global _PROGRAM
    if _PROGRAM is None:
        _PROGRAM = _build_program()
    return _PROGRAM


# ---------------------------------------------------------------- host wrapper

def prepare(source, target, mask, ln_g, ln_b, w_gq, b_gq, w_kv, b_kv, w_o1, b_o1, w_o2, h, w):
    """Build (compile-cached) program + per-core input maps from FULL inputs."""
    source = np.ascontiguousarray(np.asarray(source, dtype=np.float32))
    target = np.ascontiguousarray(np.asarray(target, dtype=np.float32))
    ln_g = np.asarray(ln_g, dtype=np.float32)
    ln_b = np.asarray(ln_b, dtype=np.float32)
    w_gq = np.asarray(w_gq, dtype=np.float32)
    b_gq = np.asarray(b_gq, dtype=np.float32)
    w_kv = np.asarray(w_kv, dtype=np.float32)
    b_kv = np.asarray(b_kv, dtype=np.float32)
    w_o1 = np.asarray(w_o1, dtype=np.float32)
    b_o1 = np.asarray(b_o1, dtype=np.float32)
    w_o2 = np.asarray(w_o2, dtype=np.float32)

    # fold LN affine into projections
    wgq_e = np.ascontiguousarray(ln_g[:, None] * w_gq)
    bgq_e = np.ascontiguousarray((b_gq + ln_b @ w_gq)[:, None])
    wkv_e = np.ascontiguousarray(ln_g[:, None] * w_kv)
    bkv_e = np.ascontiguousarray((b_kv + ln_b @ w_kv)[:, None])
    bvb = np.ascontiguousarray(np.broadcast_to(bkv_e[C:2 * C, 0][None, :], (C, C)))
    bo1_e = np.ascontiguousarray(b_o1[:, None])
    ident = np.eye(128, dtype=np.float32)
    onescol = np.ones((128, 1), dtype=np.float32)
    onerow = np.ones((1, 128), dtype=np.float32)

    nc = _get_program()

    in_maps = []
    for core in range(8):
        b = core // 2
        in_maps.append({
            "xq": np.ascontiguousarray(source[b, _Q_IDX[core]]),
            "xkv": np.ascontiguousarray(target[b, _KV_IDX[core]]),
            "wgq": wgq_e, "wkv": wkv_e, "wo1": np.ascontiguousarray(w_o1),
            "wo2": np.ascontiguousarray(w_o2), "bgq": bgq_e, "bkv": bkv_e,
            "bo1": bo1_e, "bvb": bvb, "ident": ident,
            "onescol": onescol, "onerow": onerow,
            "epsc": np.full((128, 1), EPS, dtype=np.float32),
        })
    return nc, in_maps


def unshard(per_core_y, inputs=None):
    """[8, NQ, C] per-core outputs -> full [B, SEQ, C]."""
    y = np.zeros((B, SEQ, C), dtype=np.float32)
    for core in range(8):
        b = core // 2
        y[b, _Q_IDX[core]] = per_core_y[core]
    return y


def kernel(source, target, mask, ln_g, ln_b, w_gq, b_gq, w_kv, b_kv, w_o1, b_o1, w_o2, h, w,
           _want_results=False, _trace=False):
    from concourse.bass_utils import run_bass_kernel_spmd

    nc, in_maps = prepare(source, target, mask, ln_g, ln_b, w_gq, b_gq, w_kv, b_kv,
                          w_o1, b_o1, w_o2, h, w)
    res = run_bass_kernel_spmd(nc, in_maps, list(range(8)), trace=_trace)

    y = unshard([res.results[core]["y"] for core in range(8)])
    if _want_results:
        return (y, y), res
    return (y, y)



# revision 19
# speedup vs baseline: 1.0688x; 1.0688x over previous
"""Trainium2 Bass kernel for nn_GatedAttentionUnit (Swin windowed gated attention).

Self-contained: takes FULL inputs, shards across 8 NeuronCores, returns FULL output.

Strategy
--------
The reference computes, per batch: LN -> gate/Q and K/V projections (SiLU),
Swin shifted-window attention over 16 windows (2304 tokens each) with the
standard shift mask, merge+unroll, multiplicative gate, 2-layer output MLP,
residual.

Key structural facts exploited here:
1. roll + window-split + region-sort is a pure token permutation, and every op
   outside the attention matmuls is per-token => the permutation is applied on
   the HOST to the raw inputs (a gather), and its inverse to the output.
2. The Swin shift mask makes each window's attention exactly block-diagonal
   after sorting tokens by mask region:
       win(0,0): 1x2304    win(0,1): 2x1152   win(1,0): 2x1152   win(1,1): 4x576
   so the device kernel works on dense unmasked blocks only.
3. Splitting win(0,0) by query halves gives a perfectly uniform per-core shape;
   8 cores cover batch(4) x all windows exactly.
4. LayerNorm's affine (g,b) is folded into the projection weights on the host.
5. The reference's score scale is 1/(C*seq) ~ 8.5e-7, so scaled scores x
   satisfy |x| < 1e-4 and exp(x) = 1 + x to ~1e-9 relative. Softmax is then
   EXACTLY linear in the scores, and each block's attention collapses to
       o(q) = (Vsum_b + SCALE * M_b q) / (n_b + SCALE * ksum_b . q)
   with M_b = sum_j v_j k_j^T a [C,C] matrix per block, Vsum_b = sum_j v_j,
   ksum_b = sum_j k_j. No exp, no NxN score materialization at all: the whole
   quadratic part of attention becomes 5 rank-128 collapses + per-query-chunk
   [128,qcn] matmuls. Linearization error ~6e-10 (measured) vs 2e-2 tolerance.
6. All big matmul operands are bf16 (full PE rate at any tile size, half the
   SBUF/DMA traffic); LN stats and all PSUM accumulation stay fp32.
"""

import numpy as np

# ---------------------------------------------------------------- constants
B, H, W, C, NS = 4, 96, 96, 128, 2
WH, WW = H // NS, W // NS      # 48
SH, SW = WH // 2, WW // 2      # 24
SEQ = H * W                    # 9216
NQ, NKV = 4608, 5888           # per-core query / kv tokens (kv incl. 2x64 pad for 128-alignment)
SCALE = 1.0 / float(C * SEQ)   # attention score scale
EPS = 1e-5

# (q0, k0, qn, kn) — identical block list on every core
BLOCKS = [
    (0, 0, 1152, 2304),
    (1152, 2304, 1152, 1152),
    (2304, 3456, 1152, 1152),
    (3456, 4608, 576, 576),
    (4032, 5248, 576, 576),
]

# kv 128-token tile -> (block index, valid rows)
_TILE_BLOCK = {}
for _b, (_q0, _k0, _qn, _kn) in enumerate(BLOCKS):
    _t0 = _k0 // 128
    _t1 = (_k0 + _kn + 127) // 128
    for _t in range(_t0, _t1):
        _TILE_BLOCK[_t] = (_b, min(128, _k0 + _kn - _t * 128))


def _qchunks(qn):
    if qn == 1152:
        return [384, 384, 384]
    assert qn == 576
    return [320, 256]


def _win_tokens(wy, wx):
    r = np.arange(WH)[:, None]
    c = np.arange(WW)[None, :]
    oy = (WH * wy + r + SH) % H
    ox = (WW * wx + c + SW) % W
    return oy * W + ox


def _core_index_lists():
    t00, t01, t10, t11 = (_win_tokens(0, 0), _win_tokens(0, 1),
                          _win_tokens(1, 0), _win_tokens(1, 1))
    win0_h0 = t00[:SH, :].ravel()
    win0_h1 = t00[SH:, :].ravel()
    w1a, w1b = t01[:, :SW].ravel(), t01[:, SW:].ravel()
    w2a, w2b = t10[:SH, :].ravel(), t10[SH:, :].ravel()
    w3 = [t11[:SH, :SW].ravel(), t11[:SH, SW:].ravel(),
          t11[SH:, :SW].ravel(), t11[SH:, SW:].ravel()]
    q_idx = np.zeros((8, NQ), dtype=np.int64)
    kv_idx = np.zeros((8, NKV), dtype=np.int64)
    for core in range(8):
        half = core % 2
        mine, other = (win0_h0, win0_h1) if half == 0 else (win0_h1, win0_h0)
        if half == 0:
            b1, b2, b3, b4 = w1a, w1b, w3[0], w3[1]
        else:
            b1, b2, b3, b4 = w2a, w2b, w3[2], w3[3]
        pad = np.zeros(64, dtype=b3.dtype)
        kv_idx[core] = np.concatenate([mine, other, b1, b2, b3, pad, b4, pad])
        q_idx[core] = np.concatenate([mine, b1, b2, b3, b4])
    return q_idx, kv_idx


_Q_IDX, _KV_IDX = _core_index_lists()

# ---------------------------------------------------------------- device program

_PROGRAM = None  # cached (nc,) — compile once per process


def _build_program():
    import concourse.bass as bass
    import concourse.tile as tile
    from concourse import bacc, mybir

    f32 = mybir.dt.float32
    f32r = mybir.dt.float32r
    bf16 = mybir.dt.bfloat16
    AF = mybir.ActivationFunctionType
    ts, ds = bass.ts, bass.ds

    nc = bacc.Bacc()

    # ---- DRAM parameters
    xq_d = nc.declare_dram_parameter("xq", [NQ, C], f32, isOutput=False)
    xkv_d = nc.declare_dram_parameter("xkv", [128, NKV // 128, C], bf16, isOutput=False)
    wgq_d = nc.declare_dram_parameter("wgq", [C, 2 * C], bf16, isOutput=False)
    wkv_d = nc.declare_dram_parameter("wkv", [C, 2 * C], bf16, isOutput=False)
    wo1_d = nc.declare_dram_parameter("wo1", [C, C], bf16, isOutput=False)
    wo2_d = nc.declare_dram_parameter("wo2", [C, C], bf16, isOutput=False)
    bgq_d = nc.declare_dram_parameter("bgq", [2 * C, 1], f32, isOutput=False)
    bkv_d = nc.declare_dram_parameter("bkv", [2 * C, 1], f32, isOutput=False)
    bo1_d = nc.declare_dram_parameter("bo1", [C, 1], f32, isOutput=False)
    identb_d = nc.declare_dram_parameter("identb", [128, 128], bf16, isOutput=False)
    onesb_d = nc.declare_dram_parameter("onesb", [128, 1], bf16, isOutput=False)
    eps_d = nc.declare_dram_parameter("epsc", [128, 1], f32, isOutput=False)
    onerow_d = nc.declare_dram_parameter("onerow", [1, 128], f32, isOutput=False)
    y_d = nc.declare_dram_parameter("y", [128, NQ // 128, C], bf16, isOutput=True)

    with tile.TileContext(nc) as tc:
        with (
            tc.tile_pool(name="consts", bufs=1) as cpool,
            tc.tile_pool(name="big", bufs=1) as bigpool,
            tc.tile_pool(name="xin", bufs=4) as xpool,
            tc.tile_pool(name="xnorm", bufs=4) as xnpool,
            tc.tile_pool(name="stats", bufs=6) as spool,
            tc.tile_pool(name="kvtok", bufs=6) as kvpool,
            tc.tile_pool(name="small1", bufs=3) as onepool,
            tc.tile_pool(name="t1", bufs=4) as tpool,
            tc.tile_pool(name="vt", bufs=3) as vpool,
            tc.tile_pool(name="msum", bufs=1) as mpool_sb,
            tc.tile_pool(name="yout", bufs=4) as ypool,
            tc.tile_pool(name="wk", bufs=3, space="PSUM") as wkpool,
            tc.tile_pool(name="pstp", bufs=2, space="PSUM") as tppool,
            tc.tile_pool(name="psM", bufs=1, space="PSUM") as mpool,
            tc.tile_pool(name="psS", bufs=1, space="PSUM") as rspool,
        ):
            # ---- constants into SBUF
            def cdma(shape, src, tag, dt=f32):
                t = cpool.tile(shape, dt, tag=tag)
                nc.sync.dma_start(t[:], src.bitcast(dt) if dt in (f32r,) else src)
                return t

            wgq = cdma([C, 2 * C], wgq_d[:], "wgq", bf16)
            wkv = cdma([C, 2 * C], wkv_d[:], "wkv", bf16)
            wo1 = cdma([C, C], wo1_d[:], "wo1", bf16)
            wo2 = cdma([C, C], wo2_d[:], "wo2", bf16)
            bg = cdma([C, 1], bgq_d[0:C, :], "bg")
            bq = cdma([C, 1], bgq_d[C:2 * C, :], "bq")
            bk = cdma([C, 1], bkv_d[0:C, :], "bk")
            bv = cdma([C, 1], bkv_d[C:2 * C, :], "bv")
            bo1 = cdma([C, 1], bo1_d[:], "bo1")
            identb = cdma([128, 128], identb_d[:], "identb", bf16)
            onesb = cdma([128, 1], onesb_d[:], "onesb", bf16)
            epsc = cdma([128, 1], eps_d[:], "epsc")
            onerow = cdma([1, 128], onerow_d[:], "onerow", f32r)

            # ---- big persistent SBUF tensors (tags shared across phases)
            XqT = bigpool.tile([C, NQ], bf16, tag="bigA")     # later reused as OgT
            XkvT = bigpool.tile([C, NKV], bf16, tag="bigB")   # later reused as HT
            QT = bigpool.tile([C, NQ], bf16, tag="bigC")      # later reused as Y2T
            GT = bigpool.tile([C, NQ], bf16, tag="GT")
            MT = mpool_sb.tile([128, 5 * 128], bf16, tag="MT")   # per-block (M_b)^T
            KS = mpool_sb.tile([128, 5], bf16, tag="KS")         # per-block ksum
            VS = mpool_sb.tile([128, 5], f32, tag="VS")          # per-block Vsum

            # ---- phase 1: load (4 tiles/DMA) + LN stats on DVE + transpose
            def ln_transpose(x_dram, n_tok, XT):
                nt = n_tok // 128
                for g0 in range(0, nt, 4):
                    gn = min(4, nt - g0)
                    x = xpool.tile([128, 4, C], f32, tag="x")
                    nc.sync.dma_start(
                        x[:, 0:gn, :],
                        x_dram[ds(g0 * 128, gn * 128), :].rearrange(
                            "(i p) c -> p i c", p=128))
                    st = spool.tile([128, 4, 6], f32, tag="st")
                    nc.vector.bn_stats(st[:, 0:gn, :], x[:, 0:gn, :])
                    ag = spool.tile([128, 4, 2], f32, tag="ag")
                    for i in range(gn):
                        nc.vector.bn_aggr(ag[:, i, :], st[:, i, :])
                    std = spool.tile([128, 4], f32, tag="std")
                    nc.scalar.activation(std[:, 0:gn], ag[:, 0:gn, 1:2],
                                         AF.Sqrt, bias=epsc[:])
                    rstd = spool.tile([128, 4], f32, tag="rstd")
                    nc.vector.reciprocal(rstd[:, 0:gn], std[:, 0:gn])
                    tr4 = tppool.tile([128, 4, 128], bf16, tag="tp")
                    for i in range(gn):
                        xn = xnpool.tile([128, C], bf16, tag="xn")
                        nc.vector.tensor_scalar(xn[:], x[:, i, :],
                                                ag[:, i, 0:1], rstd[:, i:i + 1],
                                                mybir.AluOpType.subtract,
                                                mybir.AluOpType.mult)
                        nc.tensor.transpose(tr4[:, i, :], xn[:], identb[:])
                    nc.gpsimd.tensor_copy(XT[:, ds(g0 * 128, gn * 128)],
                                          tr4[:, 0:gn, :])

            ln_transpose(xq_d, NQ, XqT)
            ln_transpose(xkv_d, NKV, XkvT)

            # ---- phase 2a: q-side projections (channel-major, bias+SiLU on ACT)
            def proj(wT, XT, n_tok, bias, outT, act=AF.Silu):
                off = 0
                while off < n_tok:
                    n = min(512, n_tok - off)
                    ps = wkpool.tile([128, 512], f32, tag="wk")
                    nc.tensor.matmul(ps[:, 0:n], wT,
                                     XT[:, ds(off, n)],
                                     start=True, stop=True)
                    nc.scalar.activation(outT[:, ds(off, n)], ps[:, 0:n], act, bias=bias[:])
                    off += n

            proj(wgq[:, 0:C], XqT, NQ, bg, GT)          # gate (channel-major)
            proj(wgq[:, C:2 * C], XqT, NQ, bq, QT)      # Q

            # ---- phase 2b: K/V chunks -> token-major tiles -> per-block
            # rank collapse M_b^T = sum_j k_j v_j^T, ksum_b, Vsum_b on PE.
            mall0 = mpool.tile([128, 512], f32, tag="mall0")
            mall1 = mpool.tile([128, 512], f32, tag="mall1")
            m_ps = [mall0[:, 128 * b:128 * (b + 1)] for b in range(4)]
            m_ps.append(mall1[:, 0:128])
            ks_ps = [mall1[:, 128 + b:129 + b] for b in range(5)]
            vs_ps = [mall1[:, 134 + b:135 + b] for b in range(5)]
            first_tile = {}
            last_tile = {}
            for t, (b, _jl) in _TILE_BLOCK.items():
                first_tile.setdefault(b, t)
                last_tile[b] = t

            off = 0
            while off < NKV:
                n = min(512, NKV - off)
                kps = wkpool.tile([128, 512], f32, tag="wk")
                nc.tensor.matmul(kps[:, 0:n], wkv[:, 0:C],
                                 XkvT[:, ds(off, n)], start=True, stop=True)
                kch = vpool.tile([128, 512], bf16, tag="kch")
                nc.scalar.activation(kch[:, 0:n], kps[:, 0:n], AF.Silu, bias=bk[:])
                vps = wkpool.tile([128, 512], f32, tag="wk")
                nc.tensor.matmul(vps[:, 0:n], wkv[:, C:2 * C],
                                 XkvT[:, ds(off, n)], start=True, stop=True)
                vch = vpool.tile([128, 512], bf16, tag="vch")
                nc.scalar.activation(vch[:, 0:n], vps[:, 0:n], AF.Silu, bias=bv[:])
                nt = n // 128
                ktp = tppool.tile([128, 4, 128], bf16, tag="tp")
                for k in range(nt):
                    nc.tensor.transpose(ktp[:, k, :], kch[:, ts(k, 128)], identb[:])
                ktk = kvpool.tile([128, 4, 128], bf16, tag="ktk")
                nc.gpsimd.tensor_copy(ktk[:, 0:nt, :], ktp[:, 0:nt, :])
                vtp = tppool.tile([128, 4, 128], bf16, tag="tp")
                for k in range(nt):
                    nc.tensor.transpose(vtp[:, k, :], vch[:, ts(k, 128)], identb[:])
                vtk = kvpool.tile([128, 4, 128], bf16, tag="vtk")
                nc.gpsimd.tensor_copy(vtk[:, 0:nt, :], vtp[:, 0:nt, :])
                for k in range(nt):
                    t = off // 128 + k
                    b, jlen = _TILE_BLOCK[t]
                    st_, sp_ = (t == first_tile[b]), (t == last_tile[b])
                    nc.tensor.matmul(m_ps[b], ktk[0:jlen, k, :], vtk[0:jlen, k, :],
                                     start=st_, stop=sp_)
                    nc.tensor.matmul(ks_ps[b], ktk[0:jlen, k, :], onesb[0:jlen, :],
                                     start=st_, stop=sp_)
                    nc.tensor.matmul(vs_ps[b], vtk[0:jlen, k, :], onesb[0:jlen, :],
                                     start=st_, stop=sp_)
                off += n
            for b in range(5):
                nc.gpsimd.tensor_copy(MT[:, ts(b, 128)], m_ps[b])
                nc.gpsimd.tensor_copy(KS[:, b:b + 1], ks_ps[b])
                nc.gpsimd.tensor_copy(VS[:, b:b + 1], vs_ps[b])

            # ---- phase 3: linear attention epilogue per (block, qchunk)
            # o = (Vsum + SCALE*M q) / (n + SCALE*ksum.q);  OgT = o * gate
            OgT = bigpool.tile([C, NQ], bf16, tag="bigA")  # reuses XqT slot
            for b, (q0, k0, qn, kn) in enumerate(BLOCKS):
                qc_off = 0
                for qcn in _qchunks(qn):
                    qs = q0 + qc_off
                    o_ps = wkpool.tile([128, 512], f32, tag="wk")
                    nc.tensor.matmul(o_ps[:, 0:qcn], MT[:, ts(b, 128)],
                                     QT[:, ds(qs, qcn)], start=True, stop=True)
                    rs_ps = rspool.tile([1, 512], f32, tag="rs")
                    nc.tensor.matmul(rs_ps[:, 0:qcn], KS[:, b:b + 1],
                                     QT[:, ds(qs, qcn)], start=True, stop=True)
                    d_sb = onepool.tile([1, 384], f32, tag="d")
                    nc.vector.tensor_scalar(d_sb[:, 0:qcn], rs_ps[:, 0:qcn],
                                            SCALE, float(kn),
                                            mybir.AluOpType.mult,
                                            mybir.AluOpType.add)
                    rinv = onepool.tile([1, 384], f32r, tag="rinv")
                    with nc.allow_low_precision(reason="f32r is full fp32 width"):
                        nc.vector.reciprocal(rinv[:, 0:qcn], d_sb[:, 0:qcn])
                    rb_ps = wkpool.tile([128, 512], f32, tag="wk")
                    nc.tensor.matmul(rb_ps[:, 0:qcn], onerow[:],
                                     rinv[:, 0:qcn], start=True, stop=True)
                    t2 = tpool.tile([128, 384], f32, tag="t2")
                    nc.vector.tensor_scalar(t2[:, 0:qcn], o_ps[:, 0:qcn],
                                            SCALE, VS[:, b:b + 1],
                                            mybir.AluOpType.mult,
                                            mybir.AluOpType.add)
                    t3 = tpool.tile([128, 384], f32, tag="t3")
                    nc.gpsimd.tensor_mul(t3[:, 0:qcn], t2[:, 0:qcn], rb_ps[:, 0:qcn])
                    nc.vector.tensor_mul(OgT[:, ds(qs, qcn)], t3[:, 0:qcn],
                                         GT[:, ds(qs, qcn)])
                    qc_off += qcn

            # ---- phase 4: output MLP (channel-major) + transpose + residual + store
            HT = bigpool.tile([C, NQ], bf16, tag="bigB")
            off = 0
            while off < NQ:
                n = min(512, NQ - off)
                proj_chunk(wo1, OgT, off, n, bo1, HT)
                off += n
            Y2T = bigpool.tile([C, NQ], bf16, tag="bigC")
            off = 0
            while off < NQ:
                n = min(512, NQ - off)
                ps = wkpool.tile([128, 512], f32, tag="wk")
                nc.tensor.matmul(ps[:, 0:n], wo2,
                                 HT[:, ds(off, n)],
                                 start=True, stop=True)
                nc.gpsimd.tensor_copy(Y2T[:, ds(off, n)], ps[:, 0:n])
                off += n
            for g0 in range(0, NQ // 128, 4):
                xr = xpool.tile([128, 4, C], f32, tag="x")
                nc.sync.dma_start(
                    xr[:], xq_d[ds(g0 * 128, 512), :].rearrange(
                        "(i p) c -> p i c", p=128))
                yt = ypool.tile([128, 4, C], bf16, tag="yt")
                tr4 = tppool.tile([128, 4, 128], bf16, tag="tp")
                for i in range(4):
                    nc.tensor.transpose(tr4[:, i, :], Y2T[:, ts(g0 + i, 128)], identb[:])
                nc.vector.tensor_add(yt[:], tr4[:], xr[:])
                nc.sync.dma_start(y_d[:, g0:g0 + 4, :], yt[:])

    nc.compile()
    return nc


def _get_program():
    global _PROGRAM
    if _PROGRAM is None:
        _PROGRAM = _build_program()
    return _PROGRAM


# ---------------------------------------------------------------- host wrapper

def prepare(source, target, mask, ln_g, ln_b, w_gq, b_gq, w_kv, b_kv, w_o1, b_o1, w_o2, h, w):
    """Build (compile-cached) program + per-core input maps from FULL inputs."""
    import ml_dtypes
    bf16 = ml_dtypes.bfloat16

    source = np.ascontiguousarray(np.asarray(source, dtype=np.float32))
    target = np.ascontiguousarray(np.asarray(target, dtype=np.float32))
    ln_g = np.asarray(ln_g, dtype=np.float32)
    ln_b = np.asarray(ln_b, dtype=np.float32)
    w_gq = np.asarray(w_gq, dtype=np.float32)
    b_gq = np.asarray(b_gq, dtype=np.float32)
    w_kv = np.asarray(w_kv, dtype=np.float32)
    b_kv = np.asarray(b_kv, dtype=np.float32)
    w_o1 = np.asarray(w_o1, dtype=np.float32)
    b_o1 = np.asarray(b_o1, dtype=np.float32)
    w_o2 = np.asarray(w_o2, dtype=np.float32)

    # fold LN affine into projections
    wgq_e = np.ascontiguousarray((ln_g[:, None] * w_gq).astype(bf16))
    bgq_e = np.ascontiguousarray((b_gq + ln_b @ w_gq)[:, None])
    wkv_e = np.ascontiguousarray((ln_g[:, None] * w_kv).astype(bf16))
    bkv_e = np.ascontiguousarray((b_kv + ln_b @ w_kv)[:, None])
    bo1_e = np.ascontiguousarray(b_o1[:, None])
    identb = np.eye(128, dtype=bf16)
    onesb = np.ones((128, 1), dtype=bf16)
    onerow = np.ones((1, 128), dtype=np.float32)

    nc = _get_program()

    in_maps = []
    for core in range(8):
        b = core // 2
        in_maps.append({
            "xq": np.ascontiguousarray(source[b, _Q_IDX[core]]),
            "xkv": np.ascontiguousarray(
                target[b, _KV_IDX[core]].reshape(NKV // 128, 128, C)
                .transpose(1, 0, 2).astype(bf16)),
            "wgq": wgq_e, "wkv": wkv_e,
            "wo1": np.ascontiguousarray(w_o1.astype(bf16)),
            "wo2": np.ascontiguousarray(w_o2.astype(bf16)),
            "bgq": bgq_e, "bkv": bkv_e,
            "bo1": bo1_e, "identb": identb, "onesb": onesb,
            "onerow": onerow,
            "epsc": np.full((128, 1), EPS, dtype=np.float32),
        })
    return nc, in_maps


def unshard(per_core_y, inputs=None):
    """Per-core [128, NQ//128, C] (partition-major) outputs -> full [B, SEQ, C]."""
    y = np.zeros((B, SEQ, C), dtype=np.float32)
    for core in range(8):
        b = core // 2
        yc = np.asarray(per_core_y[core])
        yc = yc.transpose(1, 0, 2).reshape(NQ, C).astype(np.float32)
        y[b, _Q_IDX[core]] = yc
    return y


def kernel(source, target, mask, ln_g, ln_b, w_gq, b_gq, w_kv, b_kv, w_o1, b_o1, w_o2, h, w,
           _want_results=False, _trace=False):
    from concourse.bass_utils import run_bass_kernel_spmd

    nc, in_maps = prepare(source, target, mask, ln_g, ln_b, w_gq, b_gq, w_kv, b_kv,
                          w_o1, b_o1, w_o2, h, w)
    res = run_bass_kernel_spmd(nc, in_maps, list(range(8)), trace=_trace)

    y = unshard([res.results[core]["y"] for core in range(8)])
    if _want_results:
        return (y, y), res
    return (y, y)
